# revision 16
# baseline (speedup 1.0000x reference)
"""Trainium2 Bass kernel for nn_CrossAttention (q-aware per-query V cross attention).

Reference computation (b=4, nq=64, n=1024, d=768, h=8, dh=96, R=64):
    q   = x @ Wq
    k   = context @ Wk
    h1  = LayerNorm(context @ Wv1)            # over the 4096 (= nq*R) axis
    vmid= h1.reshape(b, n, nq, R)
    v   = einsum('bnqr,qrd->bqnd', vmid, Wc)
    attn= softmax(q·k / sqrt(dh))             # per head
    out = einsum('bhij,bhijd->bhid', attn, v) @ Wout

Key algebraic restructuring: contract attn with vmid FIRST
(t[b,i,h,r] = sum_j attn[b,h,i,j] * vmid[b,j,i,r]), then apply the grouped
conv Wc and Wout on the tiny rank-space result. This avoids materializing
the 805MB v tensor and collapses ~52 GFLOP to ~6 GFLOP.

This version is engineered for the axon-tunnel regime where host->device
transfer (~90MB/s marginal, ~7ms fixed per array) dominates wall clock.
Per-call wire traffic is cut from ~189MB to ~14MB:
  * every replicated tensor is sharded 1/8 per core (96 = D/8 rows each of
    q^T, Wk, Wv1) and AllGathered on-device over NeuronLink. The 96-row
    shard boundaries double as the matmul contraction chunking, so the
    gathered regions are consumed in place with zero repacking;
  * one packed u8 wire tensor per core ([qT | Wk | Wv1 | ctxT]) to pay the
    per-array fixed cost once; one staging DMA + one AllGather covers the
    first three regions, ctxT stays core-local;
  * bf16 on the wire for ctx/Wk/Wv1 (q^T stays f32 as the precision
    anchor of the score path). fp8 was measured at 2.7e-2 absmax-rel for
    either ctx or Wv1 (host-only ablation matches) vs 2.5e-3 for all-bf16,
    so bf16 is the wire floor;
  * the tiny endpoints run on host: q = x@Wq (151 MFLOP) before dispatch,
    and the rank-space tail (gamma/beta fold, grouped conv Wc, Wout:
    ~330 MFLOP) after fetching the 8 ReduceScattered t slices (17KB each).

Device work per core: k head projections, h1 = ctx_loc @ Wv1 (25.8 GFLOP
fleet-wide), LN stats folded into the exp bias (e2 = exp(s)*rstd via
ln(rstd) bias; 1/rstd and mu appended as h1 columns so one matmul yields
both normalizers), t = e2^T @ h1 partial sums over the local 128 context
rows, and a ReduceScatter(add) over the query axis so each core returns
its 8 queries' totals. Host stitches the slices and finishes the tail in
numpy.
"""

import json
import os

import numpy as np
import ml_dtypes

import jax

# Fresh shard_map closures inside run_bass_kernel_spmd defeat jax's
# in-memory executable cache, so every call re-runs the BIR->NEFF pipeline
# (~0.35s). The persistent cache is keyed on the (stable) HLO hash and
# brings repeat calls down to a disk load.
try:
    jax.config.update("jax_compilation_cache_dir",
                      os.path.expanduser("~/.cache/jax_bass_cache"))
    jax.config.update("jax_persistent_cache_min_entry_size_bytes", -1)
    jax.config.update("jax_persistent_cache_min_compile_time_secs", 0.0)
except Exception:
    pass  # cache is an optimization; never fail import over it

import concourse.bass as bass
import concourse.mybir as mybir
import concourse.tile as tile
from concourse.bass_utils import run_bass_kernel_spmd

F32 = mybir.dt.float32
F32R = mybir.dt.float32r
BF16 = mybir.dt.bfloat16
AF = mybir.ActivationFunctionType
NP_BF16 = ml_dtypes.bfloat16

B = 4
NQ = 64
N = 1024
D = 768
H = 8
DH = 96
R = 64
NQR = NQ * R  # 4096
LN_EPS = 1e-5
N_CORES = 8
NLOC = N // N_CORES  # 128 context rows per batch per core
QK_SCALE = float(DH) ** -0.5

# packed wire layout per core (bytes)
QT_BYTES = DH * B * NQ * 2          # 49152  bf16 [96, 256]
WK_BYTES = DH * D * 2               # 147456 bf16 [96, 768]
WV1_BYTES = DH * NQR * 2            # 786432 bf16 [96, 4096]
GATH_BYTES = QT_BYTES + WK_BYTES + WV1_BYTES  # 1032192, AllGathered
CTX_BYTES = B * NLOC * D * 2        # 786432 bf16 [4, 128, 768], core-local
PACK_BYTES = GATH_BYTES + CTX_BYTES


class WaitSplitBass(bass.Bass):
    """This walrus build rejects instructions carrying more than one sync
    wait; split extras into preceding same-engine NoOps at JSON time."""

    MAX_WAITS = 1
    _json_memo = None

    def to_json_bytes(self) -> bytes:
        # the module is immutable once _emit() returns; serializing it costs
        # ~40ms per call inside the jit lowering, so memoize
        if self._json_memo is not None:
            return self._json_memo
        self._json_memo = self._to_json_bytes_impl()
        return self._json_memo

    def _to_json_bytes_impl(self) -> bytes:
        raw = super().to_json_bytes()
        m = json.loads(raw)
        changed = False
        for f in m.get("functions", []):
            for blk in f.get("blocks", []):
                out = []
                for inst in blk.get("instructions", []):
                    si = inst.get("sync_info")
                    waits = si.get("on_wait") if si else None
                    if waits and len(waits) > self.MAX_WAITS:
                        extra = waits[self.MAX_WAITS:]
                        si["on_wait"] = waits[: self.MAX_WAITS]
                        for k, w in enumerate(extra):
                            out.append({
                                "engine": inst["engine"],
                                "ins": [],
                                "name": f"{inst['name']}_ws{k}",
                                "opcode": "NoOp",
                                "outs": [],
                                "sync_info": {"on_update": [], "on_wait": [w]},
                            })
                        changed = True
                    out.append(inst)
                blk["instructions"] = out
        return json.dumps(m).encode() if changed else raw


def _emit(nc):
    packed = nc.declare_dram_parameter("packed", [PACK_BYTES], mybir.dt.uint8,
                                       isOutput=False)
    # ReduceScattered t sums: rows (m 2, h 8) -> query i = 16*ig + 2*core + m
    tout = nc.declare_dram_parameter("tout", [16, 4, B, 66], F32,
                                     isOutput=True)
    with tile.TileContext(nc) as tc:
        _body(nc, tc, packed, tout)
    return nc


def _view(t, byte_off, dtype, rows, cols):
    """2-D [rows, cols] view of a byte range of the flat u8 tensor t."""
    esz = np.dtype(mybir.dt.np(dtype)).itemsize
    ap = t[byte_off:byte_off + rows * cols * esz].bitcast(dtype)
    return ap.rearrange("(a b) -> a b", a=rows)


def _body(nc, tc, packed, tout):
    from contextlib import ExitStack

    with ExitStack() as st:
        const = st.enter_context(tc.tile_pool(name="const", bufs=1))
        core = st.enter_context(tc.tile_pool(name="core", bufs=1))
        small = st.enter_context(tc.tile_pool(name="small", bufs=4))
        ps_h = st.enter_context(tc.tile_pool(name="ps_h", bufs=2, space="PSUM"))
        ps_m = st.enter_context(tc.tile_pool(name="ps_m", bufs=2, space="PSUM"))
        ps_t = st.enter_context(tc.tile_pool(name="ps_t", bufs=2, space="PSUM"))
        dram = st.enter_context(tc.tile_pool(name="dram", bufs=1, space="DRAM"))

        eps_t = const.tile([128, 1], F32)
        nc.vector.memset(eps_t[:], LN_EPS)
        from concourse.masks import make_identity
        ident_bf = const.tile([128, 128], BF16)
        make_identity(nc, ident_bf[:])

        # ---- one staged copy + one AllGather of the [qT|Wk|Wv1] regions ----
        # (collectives cannot read IO tensors, hence the staging DMA)
        s_all = dram.tile([GATH_BYTES], mybir.dt.uint8)
        g_all = dram.tile([N_CORES * GATH_BYTES], mybir.dt.uint8,
                          addr_space="Shared")
        nc.sync.dma_start(out=s_all[:], in_=packed[0:GATH_BYTES])
        nc.gpsimd.collective_compute(
            "AllGather", mybir.AluOpType.bypass,
            replica_groups=[list(range(N_CORES))],
            ins=[s_all[:].opt()], outs=[g_all[:].opt()])

        # core-resident tensors
        wv1_sb = [core.tile([DH, NQR], BF16, tag=f"wv1{c}", name=f"wv1{c}")
                  for c in range(N_CORES)]
        ctxT = [core.tile([DH, B * NLOC], BF16, tag=f"cT{c}", name=f"cT{c}")
                for c in range(N_CORES)]
        q_sb = [core.tile([DH, B * NQ], F32, tag=f"q{h}", name=f"q{h}")
                for h in range(H)]
        k_sb = [core.tile([DH, B * NLOC], F32, tag=f"k{h}", name=f"k{h}")
                for h in range(H)]

        # ---- phase A: loads + k head projections ----
        with tc.tile_pool(name="phaseA", bufs=1) as pa:
            # ctx arrives in natural [b, j, d] layout (contiguous bf16 cast is
            # ~10x cheaper on host than a strided transpose-convert); the PE
            # transposes it into the 96-row contraction chunks here
            ctx_in = [pa.tile([NLOC, D], BF16, tag=f"ci{bb}", name=f"ci{bb}")
                      for bb in range(B)]
            for bb in range(B):
                nc.sync.dma_start(
                    out=ctx_in[bb][:],
                    in_=_view(packed, GATH_BYTES + bb * (NLOC * D * 2),
                              BF16, NLOC, D))
            tr_n = 0
            for c in range(N_CORES):
                for bb in range(B):
                    pt = ps_m.tile([128, 128], BF16, tag="mt", name="mt_ps")
                    nc.tensor.transpose(pt[:DH, :],
                                        ctx_in[bb][:, c * DH:(c + 1) * DH],
                                        ident_bf[:])
                    eng = nc.vector.tensor_copy if tr_n % 2 else nc.scalar.copy
                    eng(out=ctxT[c][:, bb * NLOC:(bb + 1) * NLOC],
                        in_=pt[:DH, :])
                    tr_n += 1
            wk_sb = [pa.tile([DH, D], BF16, tag=f"wk{c}", name=f"wk{c}")
                     for c in range(N_CORES)]
            q_bf = [pa.tile([DH, B * NQ], BF16, tag=f"qb{h}", name=f"qb{h}")
                    for h in range(H)]
            for c in range(N_CORES):
                base = c * GATH_BYTES
                nc.sync.dma_start(
                    out=q_bf[c][:],
                    in_=_view(g_all, base, BF16, DH, B * NQ))
                nc.sync.dma_start(
                    out=wk_sb[c][:],
                    in_=_view(g_all, base + QT_BYTES, BF16, DH, D))
                nc.sync.dma_start(
                    out=wv1_sb[c][:],
                    in_=_view(g_all, base + QT_BYTES + WK_BYTES, BF16, DH, NQR))
            # widen q to f32 so the score matmul operands match k_sb
            for h in range(H):
                eng = nc.scalar.copy if h % 2 else nc.vector.tensor_copy
                eng(out=q_sb[h][:], in_=q_bf[h][:])

            for h in range(H):
                kp = ps_m.tile([DH, B * NLOC], F32, tag="m", name="m_ps")
                for c in range(N_CORES):
                    nc.tensor.matmul(kp[:], wk_sb[c][:, h * DH:(h + 1) * DH],
                                     ctxT[c][:], start=(c == 0),
                                     stop=(c == N_CORES - 1))
                nc.scalar.copy(out=k_sb[h][:], in_=kp[:])

        # ---- phase B: h1 + attention partial sums ----
        t_all = dram.tile([128, 4, B, 66], F32)
        with tc.tile_pool(name="phaseB", bufs=1) as pb:
            # SBUF staging partitions = (i_l 4, v 32), v < 8 (= h) is live;
            # compute-engine APs must start at partition 0/32/64/96, so
            # queries sit on 32-row boundaries here and the compaction DMAs
            # below re-pack to (il, h) rows.
            t2_stage = pb.tile([128, 16, B, 66], F32, tag="t2", name="t2")

            def emit_h1(bb):
                h1_t = pb.tile([128, NQR + 2], F32R, tag=f"h1_{bb % 2}",
                               name=f"h1_{bb % 2}")
                stats = small.tile([128, 8, 6], F32, tag="stats", name="stats")
                for nn in range(8):
                    hp = ps_h.tile([128, 512], F32, tag="h_ps", name="h_ps")
                    for c in range(N_CORES):
                        nc.tensor.matmul(
                            hp[:], ctxT[c][:, bb * 128:(bb + 1) * 128],
                            wv1_sb[c][:, nn * 512:(nn + 1) * 512],
                            start=(c == 0), stop=(c == N_CORES - 1))
                    nc.vector.bn_stats(out=stats[:, nn, :], in_=hp[:])
                    nc.scalar.copy(out=h1_t[:, nn * 512:(nn + 1) * 512], in_=hp[:])
                mv = small.tile([128, 2], F32, tag="mv", name="mv")
                nc.vector.bn_aggr(out=mv[:], in_=stats[:])
                # cols 4096/4097: 1/rstd = sqrt(var+eps), mu
                nc.scalar.activation(out=h1_t[:, NQR:NQR + 1], in_=mv[:, 1:2],
                                     func=AF.Sqrt, bias=eps_t[:])
                nc.vector.tensor_copy(out=h1_t[:, NQR + 1:NQR + 2], in_=mv[:, 0:1])
                lnr = small.tile([128, 1], F32, tag="lnr", name="lnr")
                nc.scalar.activation(out=lnr[:], in_=mv[:, 1:2], func=AF.Ln,
                                     bias=eps_t[:])
                nc.vector.tensor_scalar_mul(lnr[:], lnr[:], -0.5)
                return h1_t, lnr

            def emit_scores(bb, lnr):
                # e2 col = i*32 + h (h < 8; cols h >= 8 are never-read junk)
                e2 = pb.tile([128, NQ * 32], F32R, tag="e2", name="e2")
                e2v = e2[:].rearrange("p (i v) -> p i v", v=32)
                for h in range(H):
                    sp = ps_m.tile([128, NQ], F32, tag="m", name="m_ps")
                    nc.tensor.matmul(sp[:], k_sb[h][:, bb * 128:(bb + 1) * 128],
                                     q_sb[h][:, bb * NQ:(bb + 1) * NQ],
                                     start=True, stop=True)
                    nc.scalar.activation(out=e2v[:, :, h], in_=sp[:], func=AF.Exp,
                                         scale=QK_SCALE, bias=lnr[:])
                return e2

            def emit_t5(bb, h1_t, e2):
                # t_raw chunks: 4 queries per matmul, psum partition=(i_l, v32)
                for ic in range(16):
                    tp = ps_t.tile([128, 256], F32, tag="t_ps", name="t_ps")
                    lhs = e2[:, ic * 128:(ic + 1) * 128]
                    nc.tensor.matmul(tp[:], lhs,
                                     h1_t[:, ic * 256:(ic + 1) * 256],
                                     start=True, stop=True)
                    scp = ps_m.tile([128, 2], F32, tag="m", name="m_ps")
                    nc.tensor.matmul(scp[:], lhs, h1_t[:, NQR:NQR + 2],
                                     start=True, stop=True)
                    nc.vector.tensor_copy(out=t2_stage[:, ic, bb, 64:66],
                                          in_=scp[:])
                    for il in range(4):
                        src_ap = tp[il * 32:il * 32 + 8,
                                    il * 64:(il + 1) * 64]
                        dst_ap = t2_stage[il * 32:il * 32 + 8, ic, bb, 0:64]
                        if (ic % 2) == 1:
                            nc.scalar.copy(out=dst_ap, in_=src_ap)
                        else:
                            nc.vector.tensor_copy(out=dst_ap, in_=src_ap)

            # software pipeline: PE fills the stats->exp gap of batch bb with
            # h1 matmuls of batch bb+1
            h1_cur, lnr_cur = emit_h1(0)
            e2_cur = emit_scores(0, lnr_cur)
            for bb in range(B):
                if bb + 1 < B:
                    h1_nxt, lnr_nxt = emit_h1(bb + 1)
                emit_t5(bb, h1_cur, e2_cur)
                if bb + 1 < B:
                    e2_cur = emit_scores(bb + 1, lnr_nxt)
                    h1_cur = h1_nxt

            # compact (i_l, v32) staging into (il, h) rows; plain
            # slices only (partition-split rearranges on DMA operands are
            # silently wrong on this stack)
            for ic in range(16):
                for il in range(4):
                    i = ic * 4 + il
                    row = (i % 16) * 8
                    ig = i // 16
                    nc.sync.dma_start(
                        out=t_all[row:row + 8, ig, :, :],
                        in_=t2_stage[il * 32:il * 32 + 8, ic, :, :])

        # ---- ReduceScatter over the query axis; core c owns rows 16c..16c+15,
        # i.e. queries i with i%16 in {2c, 2c+1} ----
        t_red = dram.tile([16, 4, B, 66], F32)
        nc.gpsimd.collective_compute(
            "ReduceScatter", mybir.AluOpType.add,
            replica_groups=[list(range(N_CORES))],
            ins=[t_all[:].opt()], outs=[t_red[:].opt()])
        nc.sync.dma_start(out=tout[:], in_=t_red[:])


_CACHE = {}
from concurrent.futures import ThreadPoolExecutor
_PACK_POOL = ThreadPoolExecutor(max_workers=8)
# reused across calls: run_bass_via_pjrt copies into a fresh concat array
# before transfer, so mutating these afterwards is safe
_WIRE_BUFS = [np.empty(PACK_BYTES, dtype=np.uint8) for _ in range(N_CORES)]


def _get_nc():
    if "nc" not in _CACHE:
        nc = WaitSplitBass("TRN2", target_bir_lowering=False, debug=False,
                           num_devices=N_CORES)
        _CACHE["nc"] = _emit(nc)
    return _CACHE["nc"]


def make_in_maps(x, context, Wq, Wk, Wv1, ln_g, ln_b, Wc, Wout):
    x2 = np.ascontiguousarray(x, dtype=np.float32).reshape(B * NQ, D)
    Wq = np.asarray(Wq, dtype=np.float32)
    qT = (x2 @ Wq).T  # [D, B*NQ] f32 view
    Wk = np.asarray(Wk, dtype=np.float32)
    Wv1 = np.asarray(Wv1, dtype=np.float32)
    context = np.asarray(context, dtype=np.float32)

    def pack(c):
        # convert each element exactly once, in place in the wire buffer;
        # every source slice here is (nearly) contiguous so the bf16 cast
        # runs at memory speed
        buf = _WIRE_BUFS[c]
        r = c * DH
        o = 0
        for src_arr, nbytes in (
                (qT[r:r + DH], QT_BYTES),
                (Wk[r:r + DH], WK_BYTES),
                (Wv1[r:r + DH], WV1_BYTES),
                (context[:, c * NLOC:(c + 1) * NLOC, :], CTX_BYTES)):
            v = buf[o:o + nbytes].view(NP_BF16).reshape(src_arr.shape)
            v[...] = src_arr
            o += nbytes
        assert o == PACK_BYTES
        return {"packed": buf}

    maps = list(_PACK_POOL.map(pack, range(N_CORES)))
    return maps


def assemble(results, ln_g, ln_b, Wc, Wout):
    # stitch the 8 ReduceScattered slices: core c rows = (m 2, h 8) for
    # queries i = 16*ig + 2c + m
    T = np.empty((8, 2, H, 4, B, 66), dtype=np.float32)
    for c in range(N_CORES):
        T[c] = results[c]["tout"].reshape(2, H, 4, B, 66)
    t_raw = T[..., 0:64]                       # sum_j e2 * h1_raw
    se = T[..., 64:65]                         # sum_j exp(s)
    sm = T[..., 65:66]                         # sum_j e2 * mu
    tn = (t_raw - sm) / se                     # sum_j attn * h1_norm
    # [c, m, h, ig, b, r] -> [b, h, (ig, c, m) = i, r]
    tn = np.ascontiguousarray(tn.transpose(4, 2, 3, 0, 1, 5)).reshape(
        B, H, NQ, R)
    g2 = np.asarray(ln_g, dtype=np.float32).reshape(NQ, R)
    b2 = np.asarray(ln_b, dtype=np.float32).reshape(NQ, R)
    mid = tn * g2[None, None] + b2[None, None]
    Wc4 = np.asarray(Wc, dtype=np.float32).reshape(NQ, R, H, DH)
    o = np.einsum("bhir,irhc->bihc", mid, Wc4, optimize=True).reshape(B, NQ, D)
    y = o @ np.asarray(Wout, dtype=np.float32)
    return y.astype(np.float32)


def kernel(x, context, Wq, Wk, Wv1, ln_g, ln_b, Wc, Wout):
    nc = _get_nc()
    maps = make_in_maps(x, context, Wq, Wk, Wv1, ln_g, ln_b, Wc, Wout)
    res = run_bass_kernel_spmd(nc, maps, list(range(N_CORES)))
    # guard against a transient bad result (sumexp is mathematically a sum
    # of exponentials, so any non-positive or non-finite value means the
    # dispatch glitched); re-dispatch once rather than emit NaN/garbage
    def _ok(rs):
        return all(np.isfinite(r["tout"]).all()
                   and np.all(r["tout"][:, :, :, 64] > 0) for r in rs)
    if not _ok(res.results):
        res = run_bass_kernel_spmd(nc, maps, list(range(N_CORES)))
    return assemble(res.results, ln_g, ln_b, Wc, Wout)
